# revision 67
# baseline (speedup 1.0000x reference)
"""GNN message-passing (particle simulator) Trainium2 kernel.

Strategy (8 NeuronCores, SPMD):
  - Nodes sharded 8 ways by contiguous id range (3840/core, padded).
  - Edges sharded by RECEIVER: each core owns the edges whose receiver is in
    its node shard; edges sorted by receiver and grouped into 128-receiver
    blocks (padded to a static per-block tile count so the program is SPMD).
  - Per layer: sender vectors gathered with dma_gather (transpose mode, bf16,
    feature-major) from the AllGather replica; receiver vectors gathered from
    the LOCAL cc_in shard so they overlap the collective.  Edge MLP on PE
    (bf16, fp32 PSUM); LayerNorm token-major via per-tile bn_stats with
    rstd = one ACT abs_reciprocal_sqrt op (same LUT set as relu/copy/identity
    -> zero activation-table reloads in steady state); LN apply split across
    ACT (Identity, scale/bias columns) and DVE; per-group batched residual
    adds; segment-sum via one-hot matmuls (one-hot built on GpSimd) into
    per-block PSUM accumulators; node MLP; single AllGather of the bf16 node
    shard (Shared scratchpad output).
  - Engine balance: the four 128x128 PE transposes of each edge group land in
    ONE PSUM bank and are evacuated by one batched copy; PSUM banks split
    h1:2 h2:1 h3:2 transpose:1 agg:2.
  - Residual streams: nodes fp32 resident in SBUF, edges bf16 resident.
  - Decoder on-device; Euler integration on host.
  - ALL inputs are packed into ONE bf16 blob per core and carved up on-device
    with manual access patterns (f32/i16 payloads stored as raw bits, read
    back via AP.bitcast).  The PJRT-over-axon dispatch costs ~1.25 ms per
    input BUFFER per call while bytes are nearly free, so 1 input instead of
    ~57 cuts ~60 ms/call of pure dispatch overhead (measured: 64 x 64KB
    inputs -> 87 ms/call, 1 x 4MB -> 7 ms).
  - Scatter one-hot builds alternate between GpSimd and DVE so neither
    queue stalls the PE scatter matmuls.
"""

import sys

sys.path.insert(0, "/opt/trn_rl_repo")

import numpy as np
import ml_dtypes

import concourse.bass as bass
import concourse.tile as tile
from concourse import mybir, bacc
from concourse import bass_utils

F32 = mybir.dt.float32
BF16 = mybir.dt.bfloat16
I16 = mybir.dt.int16
AF = mybir.ActivationFunctionType
ALU = mybir.AluOpType

RADIUS = 0.05


# ---------------------------------------------------------------------------
# Host-side preprocessing
# ---------------------------------------------------------------------------

def _prep(inputs, ncore, h):
    """Build per-core device arrays from the full problem inputs."""
    vel = np.asarray(inputs["vel"], np.float32)
    pos = np.asarray(inputs["pos"], np.float32)
    bounds = np.asarray(inputs["bounds"], np.float32)
    ptype = np.asarray(inputs["particle_type"], np.int32)
    senders = np.asarray(inputs["senders"], np.int32)
    receivers = np.asarray(inputs["receivers"], np.int32)
    emb = np.asarray(inputs["emb"], np.float32)

    n = pos.shape[0]
    e = senders.shape[0]

    # node features [N, 30]
    vels = vel.reshape(n, -1)
    dtw = np.concatenate([pos - bounds[:, 0], bounds[:, 1] - pos], axis=1)
    dtw = np.clip(dtw / RADIUS, -1.0, 1.0)
    type_emb = emb[ptype]
    node_attr = np.concatenate([vels, dtw, type_emb], axis=1)  # [N, 30]

    # edge features [E, 3]
    dv = (pos[senders] - pos[receivers]) / RADIUS
    dist = np.linalg.norm(dv, axis=1, keepdims=True)
    edge_attr = np.concatenate([dv, dist], axis=1)

    # shard sizing
    sh = -(-n // (128 * ncore)) * 128          # nodes per shard, mult of 128
    n_pad = sh * ncore
    nblk = sh // 128

    # AllGather pieces: the per-layer AG is split into npiece collectives so
    # each piece can start as soon as its node blocks are updated.  cc_out is
    # laid out piece-major: [piece][core][row-within-piece].
    pc_blk = 4                                  # blocks per piece
    npiece = -(-nblk // pc_blk)
    hp = [min(pc_blk, nblk - p * pc_blk) * 128 for p in range(npiece)]
    off_p = np.concatenate([[0], np.cumsum(hp)]).astype(int)   # row offset in shard
    base_p = (8 * off_p).astype(int)            # row offset in cc_out

    def cc_row(ids):
        """global node id -> row in piece-major cc_out replica"""
        ids = np.asarray(ids, np.int64)
        c, r = ids // sh, ids % sh
        p = np.searchsorted(off_p, r, side="right") - 1
        return base_p[p] + c * np.asarray(hp)[p] + (r - off_p[p])

    # sort edges by receiver, partition by receiver shard
    order = np.argsort(receivers, kind="stable")
    s_s = senders[order]
    s_r = receivers[order]
    s_a = edge_attr[order]

    core_lo = np.searchsorted(s_r, np.arange(ncore) * sh)
    core_hi = np.append(core_lo[1:], e)

    # per (core, block) edge counts -> static per-block tile budget
    counts = np.zeros((ncore, nblk), np.int64)
    for c in range(ncore):
        rloc_c = s_r[core_lo[c]:core_hi[c]] - c * sh
        blk = rloc_c // 128
        cnt = np.bincount(blk, minlength=nblk)
        counts[c] = cnt[:nblk]
    tb = np.maximum(1, -(-counts.max(axis=0) // 128))  # tiles per block
    tb[-1] += (-tb.sum()) % 4                          # total tiles mult of 4
    nt = int(tb.sum())
    e_pad = nt * 128
    blk_start = np.concatenate([[0], np.cumsum(tb)])   # tile index per block

    # per-core padded edge arrays
    per_core = []
    for c in range(ncore):
        lo, hi = core_lo[c], core_hi[c]
        rloc_c = s_r[lo:hi] - c * sh
        blk = rloc_c // 128
        bs = np.searchsorted(blk, np.arange(nblk))
        be = np.append(bs[1:], hi - lo)

        snd = np.zeros(e_pad, np.int16)
        rcv = np.zeros(e_pad, np.int16)
        rloc = np.full(e_pad, 1000.0, np.float32)
        eat = np.zeros((e_pad, 3), np.float32)
        for b in range(nblk):
            cnt = be[b] - bs[b]
            o = int(blk_start[b]) * 128
            # senders indexed in the PIECE-MAJOR cc_out layout so the per-layer
            # AllGather can be split into overlapped piece collectives
            snd[o:o + cnt] = cc_row(s_s[lo + bs[b]:lo + be[b]]).astype(np.int16)
            # receivers stored SHARD-LOCAL: gathered from this core's cc_in
            rcv[o:o + cnt] = s_r[lo + bs[b]:lo + be[b]] - c * sh
            rloc[o:o + cnt] = (rloc_c[bs[b]:be[b]] - b * 128).astype(np.float32)
            eat[o:o + cnt] = s_a[lo + bs[b]:lo + be[b]]
        per_core.append((snd, rcv, rloc, eat))

    # gather index batches: 32 tiles (4096 edges) per batch, senders then
    # receivers concatenated, wrapped [16, n/16] replicated to 128 partitions
    batch_tiles = 32
    nbatch = -(-nt // batch_tiles)
    idx_cols = []
    for k in range(nbatch):
        t0 = k * batch_tiles
        t1 = min(nt, t0 + batch_tiles)
        idx_cols.append(2 * (t1 - t0) * 128 // 16)
    idx_col_off = np.concatenate([[0], np.cumsum(idx_cols)]).astype(int)
    idxc = int(idx_col_off[-1])

    pidx_all = []
    rloc_all = []
    eat_all = []
    nat_all = []
    for c in range(ncore):
        snd, rcv, rloc, eat = per_core[c]
        pidx = np.zeros((16, idxc), np.int16)
        for k in range(nbatch):
            t0 = k * batch_tiles
            t1 = min(nt, t0 + batch_tiles)
            ne_b = (t1 - t0) * 128
            chunk = np.concatenate([snd[t0 * 128:t1 * 128], rcv[t0 * 128:t1 * 128]])
            pidx[:, idx_col_off[k]:idx_col_off[k + 1]] = (
                chunk.reshape(-1, 16).T)
        pidx_all.append(np.tile(pidx, (8, 1)))                    # [128, idxc]
        rloc_all.append(rloc.reshape(nt, 128).T.copy())           # [128, nt]
        eat_all.append(eat.T.astype(ml_dtypes.bfloat16).copy())   # [3, e_pad]

        na = np.zeros((sh, node_attr.shape[1]), np.float32)
        lo_n = c * sh
        hi_n = min(n, lo_n + sh)
        if hi_n > lo_n:
            na[:hi_n - lo_n] = node_attr[lo_n:hi_n]
        nat_all.append(na.T.astype(ml_dtypes.bfloat16).copy())    # [30, sh]

    meta = dict(
        n=n, e=e, sh=sh, n_pad=n_pad, nblk=nblk, nt=nt, e_pad=e_pad,
        tb=tuple(int(x) for x in tb),
        blk_start=tuple(int(x) for x in blk_start),
        batch_tiles=batch_tiles, nbatch=nbatch,
        idx_col_off=tuple(int(x) for x in idx_col_off),
        idxc=idxc, node_in=node_attr.shape[1],
        hp=tuple(int(x) for x in hp),
        off_p=tuple(int(x) for x in off_p),
        base_p=tuple(int(x) for x in base_p),
    )
    data = dict(pidx=pidx_all, rloc=rloc_all, eat=eat_all, nat=nat_all)
    return meta, data


# ---------------------------------------------------------------------------
# Input packing: every per-core array + all weights go into ONE bf16 blob.
# The PJRT-over-axon dispatch costs ~1.25 ms per input BUFFER per call
# (bytes are nearly free), so the kernel takes exactly 1 input instead of
# ~57.  f32 and int16 payloads are stored as raw bits and read back through
# AP.bitcast; iota/rloc are stored as true bf16 values (0..127 are exact).
# ---------------------------------------------------------------------------

def _align(x, a=256):
    return -(-x // a) * a


def _pack_blobs(meta, data, wnp, ncore):
    import ml_dtypes
    bf = ml_dtypes.bfloat16

    sh, nt, e_pad, idxc = meta["sh"], meta["nt"], meta["e_pad"], meta["idxc"]
    node_in = meta["node_in"]

    entries = [("nat", node_in * sh), ("eat", 3 * e_pad),
               ("ident", 128 * 128), ("iota", 128 * 128),
               ("rloc", 2 * 128 * nt), ("identf", 2 * 128 * 128),
               ("pidx", 128 * idxc)]
    for nm in sorted(wnp):
        n = int(np.prod(wnp[nm].shape))
        entries.append((nm, n if wnp[nm].dtype != np.float32 else 2 * n))
    offs = {}
    off = 0
    for name, nelem in entries:
        offs[name] = off
        off = _align(off + nelem)
    blen = _align(off)
    reg = {"off": offs, "len": blen}

    ident = np.eye(128, dtype=bf)
    identf = np.eye(128, dtype=np.float32)
    iota = np.tile(np.arange(128, dtype=bf)[None, :], (128, 1))

    blobs = []
    for c in range(ncore):
        b = np.zeros(blen, bf)

        def put(name, arr):
            v = arr.reshape(-1)
            if v.dtype == np.float32:
                v = v.view(bf)
            elif v.dtype == np.int16:
                v = v.view(bf)
            b[offs[name]:offs[name] + v.size] = v

        put("nat", data["nat"][c])
        put("eat", data["eat"][c])
        put("ident", ident)
        put("iota", iota)
        put("rloc", data["rloc"][c])
        put("identf", identf)
        put("pidx", data["pidx"][c])
        for nm in sorted(wnp):
            put(nm, wnp[nm])
        blobs.append(b)

    return reg, blobs


def _weights_np(inputs, L):
    """Weights in device layouts (bf16 matrices, f32 bias columns)."""
    bf = ml_dtypes.bfloat16
    w = {}
    for pre in ("ne", "ee", "de"):
        for i in (1, 2, 3):
            w[f"{pre}_W{i}"] = np.asarray(inputs[f"{pre}_W{i}"]).astype(bf)
    for pre in ("ge", "gn"):
        for i in (1, 2, 3):
            w[f"{pre}_W{i}"] = np.asarray(inputs[f"{pre}_W{i}"]).astype(bf)
    # biases / LN affine (float32)
    for pre in ("ne", "ee", "ge", "gn", "de"):
        for nm in ("b1", "b2", "b3", "g", "be"):
            key = f"{pre}_{nm}"
            if key in inputs:
                w[key] = np.asarray(inputs[key], np.float32)
    return w


# ---------------------------------------------------------------------------
# Device program
# ---------------------------------------------------------------------------

def _build(meta, wnp, L, h=128, dbg=False, reg=None):
    nc = bacc.Bacc("TRN2", target_bir_lowering=False, debug=False, num_devices=8)
    sh, n_pad, nblk, nt, e_pad = (
        meta["sh"], meta["n_pad"], meta["nblk"], meta["nt"], meta["e_pad"])
    tb, blk_start = meta["tb"], meta["blk_start"]
    nbatch, batch_tiles = meta["nbatch"], meta["batch_tiles"]
    idx_col_off, idxc = meta["idx_col_off"], meta["idxc"]
    node_in = meta["node_in"]
    ngrp_e = nt // 4
    # node-phase group widths (tiles of 128 tokens, groups of <=4 tiles)
    node_groups = []
    t0 = 0
    while t0 < nblk:
        t1 = min(nblk, t0 + 4)
        node_groups.append((t0, t1 - t0))
        t0 = t1

    # ---- I/O: ONE packed bf16 blob ----------------------------------------
    blob_bf = nc.dram_tensor("blob_bf", [reg["len"]], BF16, kind="ExternalInput")

    def _bap(off, ap):
        base = blob_bf[:]
        return bass.AP(tensor=base.tensor, offset=base.offset + off, ap=ap)

    def B_bf(name, ap, extra=0):
        return _bap(reg["off"][name] + extra, ap)

    def B_f32(name, ap, extra=0):
        # `ap` is given in f32 units (last dim must be stride-1); build the
        # doubled bf16 AP over the raw bits and bitcast back to f32.
        assert ap[-1][0] == 1
        ap2 = [[2 * s, n] for s, n in ap[:-1]] + [[1, 2 * ap[-1][1]]]
        return _bap(reg["off"][name] + 2 * extra, ap2).bitcast(F32)

    def B_i16(name, ap, extra=0):
        return _bap(reg["off"][name] + extra, ap).bitcast(I16)

    acc_out = nc.dram_tensor("acc", [sh, 2], F32, kind="ExternalOutput")
    dbg_t = {}
    if dbg:
        dbg_t["d_enc_nodes"] = nc.dram_tensor("d_enc_nodes", [128, meta["nblk"], 128], F32, kind="ExternalOutput")
        dbg_t["d_ccout0"] = nc.dram_tensor("d_ccout0", [meta["n_pad"], 128], BF16, kind="ExternalOutput")
        dbg_t["d_enc_edges"] = nc.dram_tensor("d_enc_edges", [128, meta["nt"], 128], BF16, kind="ExternalOutput")
        dbg_t["d_gbuf0"] = nc.dram_tensor("d_gbuf0", [128, 8192], BF16, kind="ExternalOutput")
        dbg_t["d_edges_l0"] = nc.dram_tensor("d_edges_l0", [128, meta["nt"], 128], BF16, kind="ExternalOutput")
        dbg_t["d_agg_l0"] = nc.dram_tensor("d_agg_l0", [128, meta["nblk"], 128], BF16, kind="ExternalOutput")
        dbg_t["d_nodes_l0"] = nc.dram_tensor("d_nodes_l0", [128, meta["nblk"], 128], F32, kind="ExternalOutput")

    use = {k: bool(np.any(np.abs(wnp[k]) > 0)) if k in wnp else False
           for k in [f"{p}_{n}" for p in ("ne", "ee", "ge", "gn", "de")
                     for n in ("b1", "b2", "b3", "be")]}
    for p in ("ne", "ee", "ge", "gn"):
        use[f"{p}_g"] = bool(np.any(np.abs(wnp[f"{p}_g"] - 1.0) > 0))

    with tile.TileContext(nc) as tc:
        # ---- pools --------------------------------------------------------
        persist = tc.alloc_tile_pool(name="persist", bufs=1)
        wpool = tc.alloc_tile_pool(name="wpool", bufs=2)
        gpool = tc.alloc_tile_pool(name="gpool", bufs=2)
        fmpool = tc.alloc_tile_pool(name="fmpool", bufs=2)
        hpool = tc.alloc_tile_pool(name="hpool", bufs=3)
        lnpool = tc.alloc_tile_pool(name="lnpool", bufs=3)
        spool = tc.alloc_tile_pool(name="spool", bufs=4)
        stage = tc.alloc_tile_pool(name="stage", bufs=3)
        ps_h1 = tc.alloc_tile_pool(name="ps_h1", bufs=2, space="PSUM")
        ps_h2 = tc.alloc_tile_pool(name="ps_h2", bufs=1, space="PSUM")
        ps_h3 = tc.alloc_tile_pool(name="ps_h3", bufs=2, space="PSUM")
        ps_t = tc.alloc_tile_pool(name="ps_t", bufs=1, space="PSUM")
        ps_agg = tc.alloc_tile_pool(name="ps_agg", bufs=2, space="PSUM")
        dram = tc.alloc_tile_pool(name="dram", bufs=1, space="DRAM")

        # ---- persistent state --------------------------------------------
        edges_res = persist.tile([128, nt, 128], BF16, tag="edges_res")
        nodes_res = persist.tile([128, nblk, 128], F32, tag="nodes_res")
        nodes_bf = persist.tile([128, nblk, 128], BF16, tag="nodes_bf")
        agg_sb = persist.tile([128, nblk, 128], BF16, tag="agg_sb")
        nodes_fm = persist.tile([128, nblk, 128], BF16, tag="nodes_fm")
        rloc_sb = persist.tile([128, nt], F32, tag="rloc_sb")
        pidx_sb = persist.tile([128, idxc], I16, tag="pidx_sb")
        ident = persist.tile([128, 128], BF16, tag="ident")
        identf = persist.tile([128, 128], F32, tag="identf")
        iota = persist.tile([128, 128], BF16, tag="iota")
        eps_col = persist.tile([128, 1], F32, tag="eps")

        nc.sync.dma_start(out=rloc_sb[:], in_=B_f32("rloc", [[nt, 128], [1, nt]]))
        nc.sync.dma_start(out=pidx_sb[:],
                          in_=B_i16("pidx", [[idxc, 128], [1, idxc]]))
        nc.sync.dma_start(out=ident[:], in_=B_bf("ident", [[128, 128], [1, 128]]))
        nc.sync.dma_start(out=identf[:], in_=B_f32("identf", [[128, 128], [1, 128]]))
        nc.sync.dma_start(out=iota[:], in_=B_bf("iota", [[128, 128], [1, 128]]))
        nc.vector.memset(eps_col[:], 1e-5)

        def load_w(name, shape, tag, dt=BF16, pool=None):
            t = (pool or wpool).tile(shape, dt, tag=tag)
            nc.sync.dma_start(
                out=t[:], in_=B_bf(name, [[shape[1], shape[0]], [1, shape[1]]]))
            return t

        def bias_col(name, l=None):
            """[dim] f32 (optionally indexed by layer) -> [dim, 1] SBUF col."""
            if not use[name]:
                return None
            dim = wnp[name].shape[-1]
            t = wpool.tile([dim, 1], F32, tag=name + "_col")
            nc.sync.dma_start(
                out=t[:],
                in_=B_f32(name, [[1, dim], [1, 1]], extra=(l or 0) * dim))
            return t

        def bcast_row(name, l=None):
            """[dim] f32 -> [128, dim] broadcast tile (for per-feature ops)."""
            dim = wnp[name].shape[-1]
            t = wpool.tile([128, dim], F32, tag=name + "_bc")
            nc.sync.dma_start(
                out=t[:],
                in_=B_f32(name, [[0, 128], [1, dim]], extra=(l or 0) * dim))
            return t

        # encoder / decoder weights resident
        ne_W1 = load_w("ne_W1", [node_in, 128], "ne_W1", pool=persist)
        ne_W2 = load_w("ne_W2", [128, 128], "ne_W2", pool=persist)
        ne_W3 = load_w("ne_W3", [128, 128], "ne_W3", pool=persist)
        ee_W1 = load_w("ee_W1", [3, 128], "ee_W1", pool=persist)
        ee_W2 = load_w("ee_W2", [128, 128], "ee_W2", pool=persist)
        ee_W3 = load_w("ee_W3", [128, 128], "ee_W3", pool=persist)
        de_W1 = load_w("de_W1", [128, 128], "de_W1", pool=persist)
        de_W2 = load_w("de_W2", [128, 128], "de_W2", pool=persist)
        de_W3 = load_w("de_W3", [128, 2], "de_W3", pool=persist)

        # DRAM ping-pong buffers for node state + collectives
        cc_in = [dram.tile([sh, 128], BF16, tag=f"ccin{l}", name=f"ccin{l}") for l in range(L + 1)]
        cc_out = [dram.tile([n_pad, 128], BF16, tag=f"ccout{l}", name=f"ccout{l}") for l in range(L + 1)]

        # Piece-wise AllGather: cc_out is piece-major ([piece][core][row]) and
        # each piece's collective fires as soon as its node groups are done,
        # overlapping the collective with the rest of the node phase.  Tile
        # allows only ONE writer for a Shared scratchpad, so each piece AG
        # lands in its own Shared tensor and a DRAM-to-DRAM copy stitches it
        # into the (Internal) gather source cc_out.  Sender gather indices
        # (host-built) use the same piece-major layout.
        hp_p, off_p, base_p = meta["hp"], meta["off_p"], meta["base_p"]
        npiece = len(hp_p)
        cc_outp = [[dram.tile([8 * hp_p[p], 128], BF16, tag=f"ccoutp{l}_{p}",
                              name=f"ccoutp{l}_{p}", addr_space="Shared")
                    for p in range(npiece)] for l in range(L + 1)]
        _ag_done = {}

        def ag_pieces(li, end_row):
            """Emit AG pieces of cc_in[li] fully covered by rows [0, end_row)."""
            if "coll0" in DISABLE:
                return
            if "coll" in DISABLE:
                if end_row == sh:
                    nc.sync.dma_start(out=cc_out[li][:sh], in_=cc_in[li][:])
                return
            p = _ag_done.get(li, 0)
            while p < npiece and off_p[p] + hp_p[p] <= end_row:
                nc.gpsimd.collective_compute(
                    "AllGather", ALU.bypass, replica_groups=[list(range(8))],
                    ins=[cc_in[li][off_p[p]:off_p[p] + hp_p[p]].opt()],
                    outs=[cc_outp[li][p][:].opt()])
                nc.sync.dma_start(
                    out=cc_out[li][base_p[p]:base_p[p] + 8 * hp_p[p]],
                    in_=cc_outp[li][p][:])
                p += 1
            _ag_done[li] = p

        # ------------------------------------------------------------------
        # generic MLP + LayerNorm over one group of `width` tokens
        # ------------------------------------------------------------------
        def act_reciprocal(out_ap, in_ap):
            """ACT-engine reciprocal (bass wrapper bans it for accuracy; the
            ~1e-3 LUT error is fine for this 2e-2-tolerance LayerNorm)."""
            nc.scalar.add_instruction(
                mybir.InstActivation(
                    name=nc.scalar.bass.get_next_instruction_name(),
                    func=AF.Reciprocal,
                    ins=[nc.scalar.lower_ap(in_ap),
                         mybir.ImmediateValue(dtype=mybir.dt.float32, value=0.0),
                         mybir.ImmediateValue(dtype=mybir.dt.float32, value=1.0),
                         mybir.ImmediateValue(dtype=mybir.dt.float32, value=0.0)],
                    outs=[nc.scalar.lower_ap(out_ap)],
                )
            )

        def mlp_ln(width, rhs_chunks, w1ch, w2, w3, b1, b2, b3row, g_bc, be_bc,
                   consume, tagp):
            nsub = width // 128
            h1 = ps_h1.tile([128, 512], F32, tag="h1")
            for i, (wc, rc) in enumerate(zip(w1ch, rhs_chunks)):
                nc.tensor.matmul(h1[:, :width], wc, rc,
                                 start=(i == 0), stop=(i == len(w1ch) - 1))
            h1s = hpool.tile([128, 512], BF16, tag="h1s")
            nc.scalar.activation(h1s[:, :width], h1[:, :width], AF.Relu,
                                 bias=(b1[:] if b1 is not None else 0.0))
            h2 = ps_h2.tile([128, 512], F32, tag="h2")
            nc.tensor.matmul(h2[:, :width], w2, h1s[:, :width],
                             start=True, stop=True)
            h2s = hpool.tile([128, 512], BF16, tag="h2s")
            nc.scalar.activation(h2s[:, :width], h2[:, :width], AF.Relu,
                                 bias=(b2[:] if b2 is not None else 0.0))
            h3 = ps_h3.tile([128, 4, 128], F32, tag="h3")
            for s in range(nsub):
                seg = h3[:, s, :]
                nc.tensor.matmul(seg, h2s[:, s * 128:(s + 1) * 128], w3,
                                 start=True, stop=True)
            if b3row is not None:
                for s in range(nsub):
                    seg = h3[:, s, :]
                    nc.vector.tensor_add(seg, seg, b3row[:])
            stats = spool.tile([128, 4, 6], F32, tag="stats")
            mv = spool.tile([128, 4, 2], F32, tag="mv")
            for s in range(nsub):
                nc.vector.bn_stats(stats[:, s, :], h3[:, s, :])
                nc.vector.bn_aggr(mv[:, s, :], stats[:, s, :])
            # rstd = 1/sqrt(var + eps) in ONE ACT op; abs_reciprocal_sqrt is
            # in the same LUT set as relu/copy/identity -> no table reloads.
            rstd = spool.tile([128, 4], F32, tag="rstd")
            nc.scalar.activation(rstd[:, :nsub], mv[:, :nsub, 1],
                                 AF.Abs_reciprocal_sqrt, bias=eps_col[:],
                                 scale=1.0)
            nmr = spool.tile([128, 4], F32, tag="nmr")
            nc.vector.scalar_tensor_tensor(nmr[:, :nsub], mv[:, :nsub, 0],
                                           -1.0, rstd[:, :nsub],
                                           ALU.mult, ALU.mult)
            for s in range(nsub):
                seg = h3[:, s, :]
                consume(s, seg, mv[:, s, 0:1], rstd[:, s:s + 1],
                        nmr[:, s:s + 1], g_bc, be_bc)

        def apply_ln(out_ap, seg, mean, rstd, nmr, g_bc, be_bc, dt_tmp,
                     engine="v"):
            """normalized = (seg - mean) * rstd [* g] [+ be] -> out_ap"""
            if g_bc is None and be_bc is None:
                if engine == "a":
                    nc.scalar.activation(out_ap, seg, AF.Identity,
                                         bias=nmr, scale=rstd)
                else:
                    nc.vector.tensor_scalar(out_ap, seg, mean, rstd,
                                            ALU.subtract, ALU.mult)
                return
            tmp = lnpool.tile([128, 128], dt_tmp, tag="ln_tmp")
            nc.vector.tensor_scalar(tmp[:], seg, mean, rstd,
                                    ALU.subtract, ALU.mult)
            if g_bc is not None and be_bc is not None:
                nc.vector.tensor_mul(tmp[:], tmp[:], g_bc[:])
                nc.vector.tensor_add(out_ap, tmp[:], be_bc[:])
            elif g_bc is not None:
                nc.vector.tensor_mul(out_ap, tmp[:], g_bc[:])
            else:
                nc.vector.tensor_add(out_ap, tmp[:], be_bc[:])

        # ------------------------------------------------------------------
        # Node encoder (sharded): nat -> nodes_res (f32) + cc_in[0] -> AG
        # ------------------------------------------------------------------
        ne_b1 = bias_col("ne_b1")
        ne_b2 = bias_col("ne_b2")
        ne_b3 = bcast_row("ne_b3") if use["ne_b3"] else None
        ne_g = bcast_row("ne_g") if use["ne_g"] else None
        ne_be = bcast_row("ne_be") if use["ne_be"] else None
        for (gt0, gnt) in node_groups:
            width = gnt * 128
            rhs = stage.tile([node_in, 512], BF16, tag="nat_stage")
            nc.sync.dma_start(out=rhs[:, :width],
                              in_=B_bf("nat", [[sh, node_in], [1, width]],
                                       extra=gt0 * 128))

            def consume_ne(s, seg, mean, rstd, nmr, g_bc, be_bc, gt0=gt0):
                t = gt0 + s
                apply_ln(nodes_res[:, t, :], seg, mean, rstd, nmr, g_bc, be_bc,
                         F32, engine=("a" if s % 2 else "v"))

            mlp_ln(width, [rhs[:, :width]], [ne_W1[:]], ne_W2[:], ne_W3[:],
                   ne_b1, ne_b2, ne_b3, ne_g, ne_be, consume_ne, "ne")
            nc.vector.tensor_copy(nodes_bf[:, gt0:gt0 + gnt, :],
                                  nodes_res[:, gt0:gt0 + gnt, :])
            nc.sync.dma_start(
                out=cc_in[0][:].rearrange("(t p) f -> p t f", p=128)[:, gt0:gt0 + gnt, :],
                in_=nodes_bf[:, gt0:gt0 + gnt, :])
            ag_pieces(0, (gt0 + gnt) * 128)
        if dbg:
            nc.sync.dma_start(out=dbg_t["d_enc_nodes"][:], in_=nodes_res[:])
            nc.sync.dma_start(out=dbg_t["d_ccout0"][:], in_=cc_out[0][:])

        # ------------------------------------------------------------------
        # Edge encoder: eat -> edges_res (bf16, token-major)
        # ------------------------------------------------------------------
        ee_b1 = bias_col("ee_b1")
        ee_b2 = bias_col("ee_b2")
        ee_b3 = bcast_row("ee_b3") if use["ee_b3"] else None
        ee_g = bcast_row("ee_g") if use["ee_g"] else None
        ee_be = bcast_row("ee_be") if use["ee_be"] else None
        for g in range(ngrp_e):
            rhs = stage.tile([3, 512], BF16, tag="eat_stage")
            nc.sync.dma_start(out=rhs[:],
                              in_=B_bf("eat", [[e_pad, 3], [1, 512]],
                                       extra=g * 512))

            def consume_ee(s, seg, mean, rstd, nmr, g_bc, be_bc, g=g):
                t = 4 * g + s
                apply_ln(edges_res[:, t, :], seg, mean, rstd, nmr, g_bc, be_bc,
                         BF16, engine=("a" if s % 2 else "v"))

            mlp_ln(512, [rhs[:]], [ee_W1[:]], ee_W2[:], ee_W3[:],
                   ee_b1, ee_b2, ee_b3, ee_g, ee_be, consume_ee, "ee")
        if dbg:
            nc.sync.dma_start(out=dbg_t["d_enc_edges"][:], in_=edges_res[:])

        # ------------------------------------------------------------------
        # Message-passing layers
        # ------------------------------------------------------------------
        tile_block = []
        pos_in_block = []
        for b in range(nblk):
            for p in range(tb[b]):
                tile_block.append(b)
                pos_in_block.append(p)

        for l in range(L):
            geW1 = wpool.tile([128, 3, 128], BF16, tag="geW1")
            nc.sync.dma_start(
                out=geW1[:],
                in_=B_bf("ge_W1", [[128, 128], [16384, 3], [1, 128]],
                         extra=l * 3 * 16384))
            geW2 = wpool.tile([128, 128], BF16, tag="geW2")
            nc.sync.dma_start(out=geW2[:], in_=B_bf(
                "ge_W2", [[128, 128], [1, 128]], extra=l * 16384))
            geW3 = wpool.tile([128, 128], BF16, tag="geW3")
            nc.sync.dma_start(out=geW3[:], in_=B_bf(
                "ge_W3", [[128, 128], [1, 128]], extra=l * 16384))
            gnW1 = wpool.tile([128, 2, 128], BF16, tag="gnW1")
            nc.sync.dma_start(
                out=gnW1[:],
                in_=B_bf("gn_W1", [[128, 128], [16384, 2], [1, 128]],
                         extra=l * 2 * 16384))
            gnW2 = wpool.tile([128, 128], BF16, tag="gnW2")
            nc.sync.dma_start(out=gnW2[:], in_=B_bf(
                "gn_W2", [[128, 128], [1, 128]], extra=l * 16384))
            gnW3 = wpool.tile([128, 128], BF16, tag="gnW3")
            nc.sync.dma_start(out=gnW3[:], in_=B_bf(
                "gn_W3", [[128, 128], [1, 128]], extra=l * 16384))
            ge_b1 = bias_col("ge_b1", l)
            ge_b2 = bias_col("ge_b2", l)
            ge_b3 = bcast_row("ge_b3", l) if use["ge_b3"] else None
            ge_g = bcast_row("ge_g", l) if use["ge_g"] else None
            ge_be = bcast_row("ge_be", l) if use["ge_be"] else None
            gn_b1 = bias_col("gn_b1", l)
            gn_b2 = bias_col("gn_b2", l)
            gn_b3 = bcast_row("gn_b3", l) if use["gn_b3"] else None
            gn_g = bcast_row("gn_g", l) if use["gn_g"] else None
            gn_be = bcast_row("gn_be", l) if use["gn_be"] else None

            # ---- edge phase ----
            gbufs = {}
            agg_ps = {}
            for g in range(ngrp_e):
                k, j = divmod(g, 8)
                if j == 0:
                    t0 = k * batch_tiles
                    t1 = min(nt, t0 + batch_tiles)
                    nb = (t1 - t0) * 128
                    ioff = idx_col_off[k]
                    ncol = nb // 16
                    gb = gpool.tile([128, 1, 8192], BF16, tag="gbuf")
                    if "gather0" in DISABLE:
                        nc.vector.memset(gb[:, :, :8], 0.25)
                    elif "gather" in DISABLE:
                        nc.vector.memset(gb[:, :, :2 * nb], 0.25)
                    else:
                        # receivers are local: gather from this core's cc_in
                        # (no AllGather dependency)
                        nc.gpsimd.dma_gather(
                            out_ap=gb[:, :, nb:2 * nb],
                            in_ap=cc_in[l][:],
                            idxs_ap=pidx_sb[:, ioff + ncol:ioff + 2 * ncol],
                            num_idxs=nb, num_idxs_reg=nb, elem_size=128,
                            transpose=True, single_packet=False)
                        # senders are global: gather from the AllGather result
                        nc.gpsimd.dma_gather(
                            out_ap=gb[:, :, :nb],
                            in_ap=cc_out[l][:],
                            idxs_ap=pidx_sb[:, ioff:ioff + ncol],
                            num_idxs=nb, num_idxs_reg=nb, elem_size=128,
                            transpose=True, single_packet=False)
                    gbufs[k] = (gb, nb)
                    if dbg and l == 0 and k == 0:
                        nc.sync.dma_start(out=dbg_t["d_gbuf0"][:, :2 * nb],
                                          in_=gb[:, 0, :2 * nb])

                gb, nedge_b = gbufs[k]
                # feature-major old edges via PE transpose; all 4 transposes
                # land in ONE PSUM bank so a single batched copy evacuates.
                efm = fmpool.tile([128, 512], BF16, tag="efm")
                if "transp0" in DISABLE:
                    nc.vector.memset(efm[:, :8], 0.25)
                elif "transp" in DISABLE:
                    nc.vector.memset(efm[:], 0.25)
                else:
                    tp4 = ps_t.tile([128, 4, 128], BF16, tag="tp4")
                    for s in range(4):
                        t = 4 * g + s
                        nc.tensor.transpose(tp4[:, s, :], edges_res[:, t, :],
                                            ident[:])
                    if g % 2:
                        nc.scalar.activation(
                            efm[:].rearrange("p (a b) -> p a b", b=128),
                            tp4[:], AF.Copy)
                    else:
                        nc.vector.tensor_copy(
                            efm[:].rearrange("p (a b) -> p a b", b=128), tp4[:])

                snd_fm = gb[:, 0, j * 512:(j + 1) * 512]
                rcv_fm = gb[:, 0, nedge_b + j * 512:nedge_b + (j + 1) * 512]

                ln4 = lnpool.tile([128, 4, 128], BF16, tag="ln4")

                def consume_ge(s, seg, mean, rstd, nmr, g_bc, be_bc, g=g,
                               ln4=ln4):
                    apply_ln(ln4[:, s, :], seg, mean, rstd, nmr, g_bc, be_bc,
                             BF16, engine=("a" if s % 2 else "v"))
                    if s < 3:
                        return
                    # one batched residual add for the whole group, then the
                    # four scatter matmuls on the updated edges; alternate the
                    # add between Pool and DVE to balance queue pressure
                    radd = nc.gpsimd if g % 2 else nc.vector
                    radd.tensor_add(edges_res[:, 4 * g:4 * g + 4, :],
                                    ln4[:],
                                    edges_res[:, 4 * g:4 * g + 4, :])
                    for q in range(4):
                        t = 4 * g + q
                        b = tile_block[t]
                        p = pos_in_block[t]
                        if "scatter0" in DISABLE:
                            if p == tb[b] - 1:
                                nc.vector.memset(agg_sb[:, b, :8], 0.25)
                            continue
                        if "scatter" in DISABLE:
                            if p == tb[b] - 1:
                                nc.vector.memset(agg_sb[:, b, :], 0.25)
                            continue
                        sm = spool.tile([128, 128], BF16, tag="S")
                        # alternate the one-hot build between Pool and DVE so
                        # neither engine's queue stalls the PE scatter matmul
                        eng = nc.gpsimd if t % 2 else nc.vector
                        eng.tensor_scalar(sm[:], iota[:],
                                          rloc_sb[:, t:t + 1], None,
                                          ALU.is_equal)
                        if p == 0:
                            agg_ps[b] = ps_agg.tile([128, 128], F32, tag="agg", name=f"agg{b}")
                        nc.tensor.matmul(agg_ps[b][:], edges_res[:, t, :], sm[:],
                                         start=(p == 0), stop=(p == tb[b] - 1))
                        if p == tb[b] - 1:
                            if b % 2:
                                nc.vector.tensor_copy(agg_sb[:, b, :],
                                                      agg_ps[b][:])
                            else:
                                nc.scalar.activation(agg_sb[:, b, :],
                                                     agg_ps[b][:], AF.Copy)
                            del agg_ps[b]

                # sender chunk LAST: efm/rcv matmuls (no AllGather dependency)
                # fill the PE while the AG tail + sender gather complete
                mlp_ln(512, [efm[:], rcv_fm, snd_fm],
                       [geW1[:, 0, :], geW1[:, 2, :], geW1[:, 1, :]],
                       geW2[:], geW3[:], ge_b1, ge_b2, ge_b3, ge_g, ge_be,
                       consume_ge, "ge")

            if dbg and l == 0:
                nc.sync.dma_start(out=dbg_t["d_edges_l0"][:], in_=edges_res[:])
                nc.sync.dma_start(out=dbg_t["d_agg_l0"][:], in_=agg_sb[:])
            # ---- node phase ----
            for b0 in range(0, nblk, 4):
                bn = min(4, nblk - b0)
                tp4 = ps_t.tile([128, 4, 128], F32, tag="tp4")
                for s in range(bn):
                    nc.tensor.transpose(tp4[:, s, :], nodes_res[:, b0 + s, :],
                                        identf[:])
                nc.vector.tensor_copy(nodes_fm[:, b0:b0 + bn, :],
                                      tp4[:, :bn, :])

            for (gt0, gnt) in node_groups:
                width = gnt * 128

                ln4f = lnpool.tile([128, 4, 128], F32, tag="ln4f")

                def consume_gn(s, seg, mean, rstd, nmr, g_bc, be_bc, gt0=gt0,
                               gnt=gnt, ln4f=ln4f):
                    apply_ln(ln4f[:, s, :], seg, mean, rstd, nmr, g_bc, be_bc,
                             F32, engine=("a" if s % 2 else "v"))
                    if s < gnt - 1:
                        return
                    nc.vector.tensor_add(nodes_res[:, gt0:gt0 + gnt, :],
                                         ln4f[:, :gnt, :],
                                         nodes_res[:, gt0:gt0 + gnt, :])
                    if l < L - 1:
                        nc.vector.tensor_copy(nodes_bf[:, gt0:gt0 + gnt, :],
                                              nodes_res[:, gt0:gt0 + gnt, :])

                mlp_ln(width,
                       [nodes_fm[:, gt0:gt0 + gnt, :].rearrange("p a b -> p (a b)"),
                        agg_sb[:, gt0:gt0 + gnt, :].rearrange("p a b -> p (a b)")],
                       [gnW1[:, 0, :], gnW1[:, 1, :]],
                       gnW2[:], gnW3[:], gn_b1, gn_b2, gn_b3, gn_g, gn_be,
                       consume_gn, "gn")
                if l < L - 1:
                    nc.sync.dma_start(
                        out=cc_in[l + 1][:].rearrange("(t p) f -> p t f", p=128)[:, gt0:gt0 + gnt, :],
                        in_=nodes_bf[:, gt0:gt0 + gnt, :])
                    ag_pieces(l + 1, (gt0 + gnt) * 128)
            if dbg and l == 0:
                nc.sync.dma_start(out=dbg_t["d_nodes_l0"][:], in_=nodes_res[:])

        # ------------------------------------------------------------------
        # Decoder + output
        # ------------------------------------------------------------------
        de_b1 = bias_col("de_b1")
        de_b2 = bias_col("de_b2")
        de_b3 = bcast_row("de_b3") if use["de_b3"] else None
        for b0 in range(0, nblk, 4):
            bn = min(4, nblk - b0)
            tp4 = ps_t.tile([128, 4, 128], F32, tag="tp4")
            for s in range(bn):
                nc.tensor.transpose(tp4[:, s, :], nodes_res[:, b0 + s, :],
                                    identf[:])
            nc.vector.tensor_copy(nodes_fm[:, b0:b0 + bn, :], tp4[:, :bn, :])
        for (gt0, gnt) in node_groups:
            width = gnt * 128
            h1 = ps_h1.tile([128, 512], F32, tag="h1")
            nc.tensor.matmul(
                h1[:, :width], de_W1[:],
                nodes_fm[:, gt0:gt0 + gnt, :].rearrange("p a b -> p (a b)"),
                start=True, stop=True)
            h1s = hpool.tile([128, 512], BF16, tag="h1s")
            nc.scalar.activation(h1s[:, :width], h1[:, :width], AF.Relu,
                                 bias=(de_b1[:] if de_b1 is not None else 0.0))
            h2 = ps_h2.tile([128, 512], F32, tag="h2")
            nc.tensor.matmul(h2[:, :width], de_W2[:], h1s[:, :width],
                             start=True, stop=True)
            h2s = hpool.tile([128, 512], BF16, tag="h2s")
            nc.scalar.activation(h2s[:, :width], h2[:, :width], AF.Relu,
                                 bias=(de_b2[:] if de_b2 is not None else 0.0))
            dec = ps_t.tile([128, 128], F32, tag="tp4")
            for s in range(gnt):
                nc.tensor.matmul(dec[:, 2 * s:2 * s + 2],
                                 h2s[:, s * 128:(s + 1) * 128], de_W3[:],
                                 start=True, stop=True)
            acc_sb = lnpool.tile([128, 8], F32, tag="acc_sb")
            if de_b3 is not None:
                for s in range(gnt):
                    nc.vector.tensor_add(acc_sb[:, 2 * s:2 * s + 2],
                                         dec[:, 2 * s:2 * s + 2], de_b3[:, :2])
            else:
                nc.vector.tensor_copy(acc_sb[:, :2 * gnt], dec[:, :2 * gnt])
            nc.sync.dma_start(
                out=acc_out[:].rearrange("(t p) c -> p t c", p=128)[:, gt0:gt0 + gnt, :],
                in_=acc_sb[:, :2 * gnt].rearrange("p (a b) -> p a b", b=2))

        for p in reversed((persist, wpool, gpool, fmpool, hpool, lnpool, spool,
                           stage, ps_h1, ps_h2, ps_h3, ps_t, ps_agg, dram)):
            p.release()

    nc.compile()
    return nc


# ---------------------------------------------------------------------------
# Entry point
# ---------------------------------------------------------------------------

_CACHE = {}
DISABLE = set()        # {"coll","gather","scatter","transp"} for HW bisect
TRACE = False          # set True (e.g. from test.py) to capture an NTFF profile
LAST_RESULT = None     # BassKernelResults of the most recent run


def make_in_maps(meta, data, wnp, ncore=8):
    reg, blobs = _pack_blobs(meta, data, wnp, ncore)
    in_maps = [{"blob_bf": blobs[c]} for c in range(ncore)]
    return reg, in_maps


def kernel(**inputs):
    ncore = 8
    h = 128
    L = int(np.asarray(inputs["ge_W1"]).shape[0])
    n = np.asarray(inputs["pos"]).shape[0]

    meta, data = _prep(inputs, ncore, h)
    wnp = _weights_np(inputs, L)
    reg, in_maps = make_in_maps(meta, data, wnp, ncore)

    cache_key = (meta["nt"], meta["tb"], meta["sh"], L)
    if cache_key not in _CACHE:
        _CACHE.clear()
        _CACHE[cache_key] = _build(meta, wnp, L, h, reg=reg)
    nc = _CACHE[cache_key]

    global LAST_RESULT
    res = bass_utils.run_bass_kernel_spmd(nc, in_maps, core_ids=list(range(ncore)),
                                          trace=TRACE)
    LAST_RESULT = res

    sh = meta["sh"]
    acc = np.zeros((n, 2), np.float32)
    for c in range(ncore):
        lo = c * sh
        hi = min(n, lo + sh)
        if hi > lo:
            acc[lo:hi] = res.results[c]["acc"][:hi - lo]

    pos = np.asarray(inputs["pos"], np.float32)
    vel = np.asarray(inputs["vel"], np.float32)
    tgt = np.asarray(inputs["tgt_pos"], np.float32)
    nonk = np.asarray(inputs["nonk_mask"], np.int32)
    pred_pos = pos + vel[:, -1, :] + acc
    pred_pos = np.where(nonk[:, None].astype(bool), pred_pos, tgt)
    return acc[:, None, :], pred_pos[:, None, :]

